# revision 15
# baseline (speedup 1.0000x reference)
"""GQA attention block (16 q heads / 2 kv heads, RoPE, causal) on 8 TRN2 NeuronCores.

Strategy: tensor-parallel over heads. Each core owns 2 q heads + the matching
kv head (kv heads replicated over 4-core groups), computes its partial o_proj
output over the full sequence, and the host sums the 8 partials. All cores run
the identical graph; only the input *data* differs per core (SPMD-safe).

Dataflow (everything "transposed" so no on-chip transpose of activations is
ever needed):
  - host passes x^T (bf16) pre-blocked per 512-seq window so every DMA is one
    contiguous read; weights are host-rearranged to [128, chunk*cols] likewise
  - scores are computed transposed: S^T[key, q] = K^T_chunk.T @ Q^T
  - softmax without max-subtraction, shifted: P = exp(s*scale - 6) on ACT
  - causal masking multiplies the diagonal-band chunks with 0/1 masks (DVE)
  - denominator: fp16 accumulation of P^T on DVE (two alternating
    accumulators for long windows to halve the serial-chain latency), the
    ones-vector matmul partition-reduce deferred to the END of the window so
    PE never stalls on the DVE chain, reciprocal_approx_fast + gpsimd
    partition_broadcast, scale folded into the out^T -> SBUF copy
  - PV accumulates out^T[d, q] with V (natural layout, via PE transpose)
    stationary and P^T streaming

Schedule: ONE loop over the 8 seq-windows. The attention group loop of
window j is the ACT-paced stretch (exp at ~1.1us per group vs 864ns of
QK+PV); the PE bubbles there are filled with "filler units": the Q/K/V
projection matmuls of window j+1 and the o_proj matmul blocks of window
j-1, distributed evenly across the groups. Projection psum evacuation +
bias ride the ACT engine (Identity activation with a [128,1] bias vector)
— cheap enough not to perturb the exp cadence; o_proj evacuation runs on
ACT for the exp-light early windows and on DVE for the late ones.

Measured: 382 us baseline -> ~345 us target here. PE is the bottleneck
(~330 us busy at the 216 ns N=512 pace); fp8 gives no per-output speedup
on TRN2 (DoubleRow only halves contraction instruction count) and its
element noise exceeds the 2e-2 error budget anyway.
"""

import os
import sys

for _p in ("/opt/trn_rl_repo",):
    if os.path.isdir(_p) and _p not in sys.path:
        sys.path.append(_p)

import numpy as np
import ml_dtypes

FP16 = np.float16
BF16 = ml_dtypes.bfloat16

# ---- problem constants (hardcoded per harness contract) ----
S = 4096          # sequence length
H = 2048          # hidden
DH = 128          # head dim
N_CORES = 8
HC = H // 128     # 16 hidden chunks
W = 512           # q-window width
NW = S // W       # 8 windows
SQ = S // 4       # sequence quarter (per-core K/V share)
SCALE = 1.0 / float(np.sqrt(DH))
EXP_SHIFT = -6.0

_CACHE = {}


def _build():
    import concourse.bacc as bacc
    import concourse.mybir as mybir
    import concourse.tile as tile
    from concourse.masks import make_identity

    dt = mybir.dt
    AF = mybir.ActivationFunctionType

    nc = bacc.Bacc("TRN2", target_bir_lowering=False, debug=False,
                   num_devices=N_CORES)

    xt = nc.dram_tensor("xt", [NW, 128, HC * W], dt.bfloat16, kind="ExternalInput")
    wq = nc.dram_tensor("wq", [128, HC * 2 * DH], dt.bfloat16, kind="ExternalInput")
    wk = nc.dram_tensor("wk", [128, HC * DH], dt.bfloat16, kind="ExternalInput")
    wv = nc.dram_tensor("wv", [128, HC * DH], dt.bfloat16, kind="ExternalInput")
    wo = nc.dram_tensor("wo", [128, 2 * H], dt.bfloat16, kind="ExternalInput")
    bqd = nc.dram_tensor("bq", [128, 2], dt.float32, kind="ExternalInput")
    bkvd = nc.dram_tensor("bkv", [128, 2], dt.float32, kind="ExternalInput")
    cosd = nc.dram_tensor("cost", [128, S], dt.bfloat16, kind="ExternalInput")
    sind = nc.dram_tensor("sins", [128, S], dt.bfloat16, kind="ExternalInput")
    mskd = nc.dram_tensor("msk", [128, 4 * W], dt.bfloat16, kind="ExternalInput")
    out = nc.dram_tensor("out", [S, H], dt.float32, kind="ExternalOutput")

    with tile.TileContext(nc) as tc:
        with (
            tc.tile_pool(name="const", bufs=1) as constp,
            tc.tile_pool(name="xtp", bufs=2) as xtp,
            tc.tile_pool(name="proj", bufs=1) as projp,
            tc.tile_pool(name="ptp", bufs=8) as ptp,
            tc.tile_pool(name="work", bufs=2) as workp,
            tc.tile_pool(name="otsp", bufs=5) as otsp,
            tc.tile_pool(name="obp", bufs=2) as obp,
            tc.tile_pool(name="pp", bufs=2, space="PSUM") as pp,
            tc.tile_pool(name="pqk", bufs=2, space="PSUM") as pqk,
            tc.tile_pool(name="ppv", bufs=2, space="PSUM") as ppv,
        ):
            # ---------- constants into SBUF ----------
            wq_sb = constp.tile([128, HC * 2 * DH], dt.bfloat16, tag="wq")
            wk_sb = constp.tile([128, HC * DH], dt.bfloat16, tag="wk")
            wv_sb = constp.tile([128, HC * DH], dt.bfloat16, tag="wv")
            wo_sb = constp.tile([128, 2 * H], dt.bfloat16, tag="wo")
            bq_sb = constp.tile([128, 2], dt.float32, tag="bq")
            bkv_sb = constp.tile([128, 2], dt.float32, tag="bkv")
            cos_sb = constp.tile([128, S], dt.bfloat16, tag="cos")
            sin_sb = constp.tile([128, S], dt.bfloat16, tag="sin")
            msk_sb = constp.tile([128, 4 * W], dt.bfloat16, tag="msk")
            ones_sb = constp.tile([128, 1], dt.float16, tag="ones")
            ident = constp.tile([128, 128], dt.bfloat16, tag="ident")
            negC = constp.tile([128, 1], dt.float32, tag="negC")

            # startup: only the window-0 critical DMAs go on the sync queue;
            # everything else queues behind on gpsimd so it doesn't steal HBM
            # bandwidth from the first projection matmuls
            nc.sync.dma_start(wk_sb[:], wk[:, :])
            nc.sync.dma_start(wq_sb[:, 0:1024], wq[:, 0:1024])
            nc.gpsimd.dma_start(bq_sb[:], bqd[:, :])
            nc.gpsimd.dma_start(bkv_sb[:], bkvd[:, :])
            nc.gpsimd.dma_start(cos_sb[:], cosd[:, :])
            nc.gpsimd.dma_start(sin_sb[:], sind[:, :])
            nc.gpsimd.dma_start(wv_sb[:], wv[:, :])
            nc.gpsimd.memset(ones_sb[:], 1.0)
            nc.gpsimd.memset(negC[:], EXP_SHIFT)
            make_identity(nc, ident[:])

            qt_sb = projp.tile([128, 2 * S], dt.bfloat16, tag="qt")
            kt_q = [projp.tile([128, SQ], dt.bfloat16, tag=f"ktq{r}",
                               name=f"ktq{r}") for r in range(4)]
            vn_q = [projp.tile([128, SQ], dt.bfloat16, tag=f"vnq{r}",
                               name=f"vnq{r}") for r in range(4)]

            def kt_chunk(k):
                return kt_q[k // 8][:, (k % 8) * 128:(k % 8 + 1) * 128]

            def vn_chunk(k):
                return vn_q[k // 8][:, (k % 8) * 128:(k % 8 + 1) * 128]

            shuffle_mask = [i ^ 1 for i in range(32)]

            def rope_store(t0, dest_slc, sb):
                tsw = workp.tile([128, W], dt.bfloat16, tag="ropesw")
                nc.vector.stream_shuffle(tsw[:], t0[:], mask=shuffle_mask)
                t1 = workp.tile([128, W], dt.bfloat16, tag="rope1")
                nc.vector.tensor_mul(t1[:], t0[:],
                                     cos_sb[:, sb * W:(sb + 1) * W])
                t2 = workp.tile([128, W], dt.bfloat16, tag="rope2")
                nc.vector.tensor_mul(t2[:], tsw[:],
                                     sin_sb[:, sb * W:(sb + 1) * W])
                nc.vector.tensor_add(dest_slc, t1[:], t2[:])

            # ---------- projection filler units for window sb ----------
            # each unit emits ~2-4 PE matmuls (plus evac side-work on
            # ACT/DVE at target boundaries); DMA for xb is issued when the
            # unit list is built
            def proj_units(sb):
                xb = xtp.tile([128, HC * W], dt.bfloat16, tag="xtb")
                if sb == 0:
                    for q4 in range(4):
                        nc.sync.dma_start(
                            xb[:, q4 * HC * W // 4:(q4 + 1) * HC * W // 4],
                            xt[0, :, q4 * HC * W // 4:(q4 + 1) * HC * W // 4])
                    nc.sync.dma_start(wq_sb[:, 1024:], wq[:, 1024:])
                targets = [
                    ("rope", lambda h: wk_sb[:, h * 128:(h + 1) * 128],
                     bkv_sb[:, 0:1], kt_q[sb // 2], (sb % 2) * W),
                    ("rope", lambda h: wq_sb[:, h * 256:h * 256 + 128],
                     bq_sb[:, 0:1], qt_sb, sb * W),
                    ("vnat", lambda h: wv_sb[:, h * 128:(h + 1) * 128],
                     bkv_sb[:, 1:2], vn_q[sb // 2], 0),
                    ("rope", lambda h: wq_sb[:, h * 256 + 128:h * 256 + 256],
                     bq_sb[:, 1:2], qt_sb, S + sb * W),
                ]
                units = []
                state = {}

                def mm_unit(kind, wslc, bias, dest, doff, h0, first_unit):
                    def emit():
                        if first_unit and sb > 0:
                            nc.sync.dma_start(xb[:], xt[sb, :, :])
                        if h0 == 0:
                            state["ps"] = pp.tile([128, W], dt.float32,
                                                  tag="pp_ps", name="ps")
                        ps = state["ps"]
                        for h in range(h0, h0 + 4):
                            nc.tensor.matmul(
                                ps[:], wslc(h), xb[:, h * W:(h + 1) * W],
                                start=(h == 0), stop=(h == HC - 1))
                        if h0 + 4 == HC:
                            t0 = workp.tile([128, W], dt.bfloat16, tag="evac0",
                                            name="t0")
                            nc.scalar.activation(t0[:], ps[:], AF.Identity,
                                                 bias=bias)
                            if kind == "rope":
                                rope_store(t0, dest[:, doff:doff + W], sb)
                            else:
                                state["vstage"] = t0
                    return emit

                def tr_unit(dest, i):
                    def emit():
                        t0 = state["vstage"]
                        tp = pp.tile([128, 128], dt.bfloat16, tag="pp_ps",
                                     name="tp")
                        nc.tensor.transpose(
                            tp[:], t0[:, i * 128:(i + 1) * 128], ident[:])
                        nc.vector.tensor_copy(
                            dest[:, ((sb % 2) * 4 + i) * 128:
                                 ((sb % 2) * 4 + i + 1) * 128],
                            tp[:])
                    return emit

                for ti, (kind, wslc, bias, dest, doff) in enumerate(targets):
                    for h0 in range(0, HC, 4):
                        units.append(mm_unit(kind, wslc, bias, dest, doff, h0,
                                             ti == 0 and h0 == 0))
                    if kind == "vnat":
                        for i in range(W // 128):
                            units.append(tr_unit(dest, i))
                return units

            # ---------- o_proj filler units for window jm1 ----------
            def oproj_units(jm1, ots_heads, evac):
                units = []

                def qc_unit(qc, n):
                    def emit():
                        if n == 0:
                            oproj_state[qc] = obp.tile([128, H], dt.float32,
                                                       tag="ob", name="ob")
                        ob = oproj_state[qc]
                        po = pp.tile([128, W], dt.float32, tag="pp_ps",
                                     name="po")
                        for a in range(2):
                            nc.tensor.matmul(
                                po[:],
                                ots_heads[a][:, qc * 128:(qc + 1) * 128],
                                wo_sb[:, a * H + n * W: a * H + (n + 1) * W],
                                start=(a == 0), stop=(a == 1))
                        eng = evac if evac in ("act", "dve") else \
                            ("act" if n % 2 == 0 else "dve")
                        if eng == "act":
                            nc.scalar.activation(ob[:, n * W:(n + 1) * W],
                                                 po[:], AF.Copy)
                        else:
                            nc.vector.tensor_copy(ob[:, n * W:(n + 1) * W],
                                                  po[:])
                        if n == H // W - 1:
                            nc.sync.dma_start(
                                out[jm1 * W + qc * 128:
                                    jm1 * W + (qc + 1) * 128, :], ob[:])
                    return emit

                oproj_state = {}
                for qc in range(4):
                    for n in range(H // W):
                        units.append(qc_unit(qc, n))
                return units

            # ---------- attention for one head, with fillers ----------
            def attn_head(a, j, fillers):
                nkc = 4 * j + 4
                split = j >= 4  # two dacc accumulators for long chains
                qslc = qt_sb[:, a * S + j * W: a * S + (j + 1) * W]
                ot = ppv.tile([128, W], dt.float32, tag="ppv_ps", name="ot")
                dacc0 = workp.tile([128, 2 * W], dt.float16, tag="dacc0",
                                   name="dacc0")
                dacc1 = (workp.tile([128, 2 * W], dt.float16, tag="dacc1",
                                    name="dacc1") if split else None)
                for g in range(nkc // 2):
                    ps = pqk.tile([128, 2 * W], dt.float32, tag="qk_ps",
                                  name="ps")
                    ptg = ptp.tile([128, 2 * W], dt.bfloat16, tag="pt",
                                   name="ptg")
                    dacc = dacc1 if (split and g % 2 == 1) else dacc0
                    first = g < 2 if split else g < 1
                    last = g == nkc // 2 - 1
                    if last:
                        # QK only over the causally-valid columns (the rest
                        # of this psum is never read by the slimmed exp)
                        nc.tensor.matmul(
                            ps[:, 256:512], kt_chunk(2 * g),
                            qslc[:, 256:512], start=True, stop=True)
                        nc.tensor.matmul(
                            ps[:, 896:1024], kt_chunk(2 * g + 1),
                            qslc[:, 384:512], start=True, stop=True)
                    else:
                        for r in range(2):
                            nc.tensor.matmul(
                                ps[:, r * W:(r + 1) * W],
                                kt_chunk(2 * g + r),
                                qslc, start=True, stop=True)
                    for f in fillers.take():
                        f()
                    if last:
                        # last group = diagonal chunks r=2,3: columns
                        # [0:256] / [512:896] are fully causal-masked, so
                        # exp/mask/dacc/PV all skip them
                        nc.scalar.activation(ptg[:, 256:512], ps[:, 256:512],
                                             AF.Exp, scale=SCALE, bias=negC[:])
                        nc.scalar.activation(ptg[:, 896:1024], ps[:, 896:1024],
                                             AF.Exp, scale=SCALE, bias=negC[:])
                        nc.vector.tensor_mul(
                            ptg[:, 256:512], ptg[:, 256:512],
                            msk_sb[:, 2 * W + 256:3 * W])
                        nc.vector.tensor_mul(
                            ptg[:, 896:1024], ptg[:, 896:1024],
                            msk_sb[:, 3 * W + 384:4 * W])
                        nc.vector.tensor_add(dacc[:, 256:512],
                                             dacc[:, 256:512],
                                             ptg[:, 256:512])
                        nc.vector.tensor_add(dacc[:, 896:1024],
                                             dacc[:, 896:1024],
                                             ptg[:, 896:1024])
                        nc.tensor.matmul(
                            ot[:, 256:512], vn_chunk(2 * g),
                            ptg[:, 256:512], start=False, stop=False)
                        nc.tensor.matmul(
                            ot[:, 384:512], vn_chunk(2 * g + 1),
                            ptg[:, 896:1024], start=False, stop=True)
                        continue
                    nc.scalar.activation(ptg[:], ps[:], AF.Exp,
                                         scale=SCALE, bias=negC[:])
                    if g == nkc // 2 - 2:
                        nc.vector.tensor_mul(
                            ptg[:], ptg[:], msk_sb[:, 0:2 * W])
                    if first:
                        nc.vector.tensor_copy(dacc[:], ptg[:])
                    else:
                        nc.vector.tensor_add(dacc[:], dacc[:], ptg[:])
                    for r in range(2):
                        k = 2 * g + r
                        nc.tensor.matmul(
                            ot[:], vn_chunk(k),
                            ptg[:, r * W:(r + 1) * W],
                            start=(k == 0), stop=False)
                return ot, dacc0, dacc1

            # window-end denominator + scale for one head -> ots tile
            def finish_head(ot, dacc0, dacc1):
                dn = pp.tile([1, W], dt.float32, tag="pp_ps", name="dn")
                segs = [dacc0[:, 0:W], dacc0[:, W:2 * W]]
                if dacc1 is not None:
                    segs += [dacc1[:, 0:W], dacc1[:, W:2 * W]]
                for i, seg in enumerate(segs):
                    nc.tensor.matmul(dn[0:1, :], ones_sb[:, 0:1], seg,
                                     start=(i == 0), stop=(i == len(segs) - 1))
                drc = workp.tile([1, W], dt.float32, tag="drc")
                nc.vector.reciprocal_approx_fast(drc[:], dn[0:1, :])
                drb = workp.tile([128, W], dt.float32, tag="drb")
                nc.gpsimd.partition_broadcast(drb[:], drc[:])
                ots = otsp.tile([128, W], dt.bfloat16, tag="ots")
                nc.vector.tensor_mul(ots[:], ot[:], drb[:])
                return ots

            class Fillers:
                """Distributes filler units evenly over `take()` calls."""

                def __init__(self, units, ntakes):
                    self.units = units
                    self.ntakes = max(ntakes, 1)
                    self.taken = 0
                    self.pos = 0

                def take(self):
                    self.taken += 1
                    end = (len(self.units) * self.taken) // self.ntakes
                    u = self.units[self.pos:end]
                    self.pos = end
                    return u

                def rest(self):
                    u = self.units[self.pos:]
                    self.pos = len(self.units)
                    return u

            # ---------- fused window loop ----------
            # window 0's projections run as a straight block (attention
            # depends on them); window j then computes attention j with
            # proj(j+1) and o_proj(j-1) as PE fillers inside the group loop
            for f in proj_units(0):
                f()
            nc.gpsimd.dma_start(msk_sb[:], mskd[:, :])
            nc.gpsimd.dma_start(wo_sb[:], wo[:, :])
            prev = None
            for j in range(NW):
                units = []
                if j + 1 < NW:
                    units += proj_units(j + 1)
                if prev is not None:
                    units += oproj_units(j - 1, prev, "mix")
                fillers = Fillers(units, 2 * (2 * j + 2))
                h0 = attn_head(0, j, fillers)
                h1 = attn_head(1, j, fillers)
                for f in fillers.rest():
                    f()
                o0 = finish_head(*h0)
                o1 = finish_head(*h1)
                prev = (o0, o1)
            units_a = oproj_units(NW - 1, prev, "act")
            units_d = oproj_units(NW - 1, prev, "dve")
            for i in range(16):
                (units_a if (i // 4) % 2 == 0 else units_d)[i]()

    nc.compile()
    return nc


def _prep_inputs(x, cos, sin, Wq, bq, Wk, bk, Wv, bv, Wo):
    x = np.asarray(x, dtype=np.float32).reshape(S, H)
    cos = np.asarray(cos, dtype=np.float32).reshape(S, DH)
    sin = np.asarray(sin, dtype=np.float32).reshape(S, DH)

    xtT = x.T.astype(BF16)                       # [H, S]
    # blocked layout: [seq_block, partition, hid_chunk * W] so each block's
    # DMA is one fully-contiguous read
    xtb = np.ascontiguousarray(
        xtT.reshape(HC, 128, NW, W).transpose(2, 1, 0, 3).reshape(NW, 128, HC * W))

    # head-dim permutation: partition 2t <- dim t, partition 2t+1 <- dim t+64
    perm = np.empty(DH, np.int64)
    perm[0::2] = np.arange(64)
    perm[1::2] = np.arange(64) + 64

    cosT = np.ascontiguousarray(cos.T)          # [128, S]
    sinT = np.ascontiguousarray(sin.T)
    cosP = np.ascontiguousarray(cosT[perm]).astype(BF16)
    sinsP = np.empty_like(sinT)
    sinsP[0::2] = -sinT[:64]
    sinsP[1::2] = sinT[:64]
    sinsP = np.ascontiguousarray(sinsP).astype(BF16)

    # causal 0/1 masks for the 4 diagonal-band chunks of each 512-q window
    kk = np.arange(128)[:, None]
    qq = np.arange(W)[None, :]
    msk = np.concatenate(
        [(qq >= kk + 128 * r).astype(np.float32) for r in range(4)],
        axis=1).astype(BF16)

    Wq = np.asarray(Wq, np.float32)
    Wk = np.asarray(Wk, np.float32)
    Wv = np.asarray(Wv, np.float32)
    Wo = np.asarray(Wo, np.float32)
    bq = np.asarray(bq, np.float32)
    bk = np.asarray(bk, np.float32)
    bv = np.asarray(bv, np.float32)

    in_maps = []
    for c in range(N_CORES):
        kv = c // 4
        # q/k projections get the RoPE head-dim permutation applied to their
        # output columns (and biases); v/o stay in natural order
        wq_c = np.concatenate(
            [Wq[:, (2 * c + a) * DH:(2 * c + a + 1) * DH][:, perm]
             for a in range(2)], axis=1)
        wk_c = Wk[:, kv * DH:(kv + 1) * DH][:, perm]
        wv_c = Wv[:, kv * DH:(kv + 1) * DH]
        wo_c = Wo[2 * c * DH:(2 * c + 2) * DH, :]
        bq_c = np.stack(
            [bq[(2 * c + a) * DH:(2 * c + a + 1) * DH][perm] for a in range(2)],
            axis=1)
        bkv_c = np.stack(
            [bk[kv * DH:(kv + 1) * DH][perm], bv[kv * DH:(kv + 1) * DH]],
            axis=1)

        def wrearr(w):
            c = w.shape[0] // 128
            return np.ascontiguousarray(
                w.reshape(c, 128, -1).transpose(1, 0, 2).reshape(128, -1))

        in_maps.append({
            "xt": xtb,
            "wq": wrearr(wq_c).astype(BF16),
            "wk": wrearr(wk_c).astype(BF16),
            "wv": wrearr(wv_c).astype(BF16),
            "wo": wrearr(wo_c).astype(BF16),
            "bq": np.ascontiguousarray(bq_c).astype(np.float32),
            "bkv": np.ascontiguousarray(bkv_c).astype(np.float32),
            "cost": cosP, "sins": sinsP,
            "msk": msk,
        })
    return in_maps


def _get_nc():
    if "nc" not in _CACHE:
        _CACHE["nc"] = _build()
    return _CACHE["nc"]


def run(trace=False, tmpdir=None, **inputs):
    from concourse.bass_utils import run_bass_kernel_spmd

    nc = _get_nc()
    in_maps = _prep_inputs(**inputs)
    kw = {}
    if trace:
        kw = dict(trace=True, tmpdir=tmpdir)
    res = run_bass_kernel_spmd(nc, in_maps, core_ids=list(range(N_CORES)), **kw)
    acc = np.zeros((S, H), dtype=np.float32)
    for r in res.results:
        acc += r["out"]
    return acc.reshape(1, S, H), res


def kernel(**inputs) -> np.ndarray:
    out, _ = run(**inputs)
    return out


# revision 16
# speedup vs baseline: 1.0271x; 1.0271x over previous
"""GQA attention block (16 q heads / 2 kv heads, RoPE, causal) on 8 TRN2 NeuronCores.

Strategy: tensor-parallel over heads. Each core owns 2 q heads + the matching
kv head (kv heads replicated over 4-core groups), computes its partial o_proj
output over the full sequence, and the host sums the 8 partials. All cores run
the identical graph; only the input *data* differs per core (SPMD-safe).

Dataflow (everything "transposed" so no on-chip transpose of activations is
ever needed):
  - host passes x^T (bf16) pre-blocked per 512-seq window so every DMA is one
    contiguous read; weights are host-rearranged to [128, chunk*cols] likewise
  - scores are computed transposed: S^T[key, q] = K^T_chunk.T @ Q^T
  - softmax without max-subtraction, shifted: P = exp(s*scale - 6) on ACT
  - causal masking multiplies the diagonal-band chunks with 0/1 masks (DVE)
  - denominator: fp16 accumulation of P^T on DVE (two alternating
    accumulators for long windows to halve the serial-chain latency), the
    ones-vector matmul partition-reduce deferred to the END of the window so
    PE never stalls on the DVE chain, reciprocal_approx_fast + gpsimd
    partition_broadcast, scale folded into the out^T -> SBUF copy
  - PV accumulates out^T[d, q] with V (natural layout, via PE transpose)
    stationary and P^T streaming

Schedule: ONE loop over the 8 seq-windows. The attention group loop of
window j is the ACT-paced stretch (exp at ~1.1us per group vs 864ns of
QK+PV); the PE bubbles there are filled with "filler units": the Q/K/V
projection matmuls of window j+1 and the o_proj matmul blocks of window
j-1, distributed evenly across the groups. Projection psum evacuation +
bias ride the ACT engine (Identity activation with a [128,1] bias vector)
— cheap enough not to perturb the exp cadence; o_proj evacuation runs on
ACT for the exp-light early windows and on DVE for the late ones.

Measured: 382 us baseline -> ~345 us target here. PE is the bottleneck
(~330 us busy at the 216 ns N=512 pace); fp8 gives no per-output speedup
on TRN2 (DoubleRow only halves contraction instruction count) and its
element noise exceeds the 2e-2 error budget anyway.
"""

import os
import sys

for _p in ("/opt/trn_rl_repo",):
    if os.path.isdir(_p) and _p not in sys.path:
        sys.path.append(_p)

import numpy as np
import ml_dtypes

FP16 = np.float16
BF16 = ml_dtypes.bfloat16

# ---- problem constants (hardcoded per harness contract) ----
S = 4096          # sequence length
H = 2048          # hidden
DH = 128          # head dim
N_CORES = 8
HC = H // 128     # 16 hidden chunks
W = 512           # q-window width
NW = S // W       # 8 windows
SQ = S // 4       # sequence quarter (per-core K/V share)
SCALE = 1.0 / float(np.sqrt(DH))
EXP_SHIFT = -6.0

_CACHE = {}


def _build():
    import concourse.bacc as bacc
    import concourse.mybir as mybir
    import concourse.tile as tile
    from concourse.masks import make_identity

    dt = mybir.dt
    AF = mybir.ActivationFunctionType

    nc = bacc.Bacc("TRN2", target_bir_lowering=False, debug=False,
                   num_devices=N_CORES)

    xt = nc.dram_tensor("xt", [NW, 128, HC * W], dt.bfloat16, kind="ExternalInput")
    wq = nc.dram_tensor("wq", [128, HC * 2 * DH], dt.bfloat16, kind="ExternalInput")
    wk = nc.dram_tensor("wk", [128, HC * DH], dt.bfloat16, kind="ExternalInput")
    wv = nc.dram_tensor("wv", [128, HC * DH], dt.bfloat16, kind="ExternalInput")
    wo = nc.dram_tensor("wo", [128, 2 * H], dt.bfloat16, kind="ExternalInput")
    bqd = nc.dram_tensor("bq", [128, 2], dt.float32, kind="ExternalInput")
    bkvd = nc.dram_tensor("bkv", [128, 2], dt.float32, kind="ExternalInput")
    cosd = nc.dram_tensor("cost", [128, S], dt.bfloat16, kind="ExternalInput")
    sind = nc.dram_tensor("sins", [128, S], dt.bfloat16, kind="ExternalInput")
    mskd = nc.dram_tensor("msk", [128, 4 * W], dt.bfloat16, kind="ExternalInput")
    out = nc.dram_tensor("out", [S, H], dt.float32, kind="ExternalOutput")

    with tile.TileContext(nc) as tc:
        with (
            tc.tile_pool(name="const", bufs=1) as constp,
            tc.tile_pool(name="xtp", bufs=2) as xtp,
            tc.tile_pool(name="proj", bufs=1) as projp,
            tc.tile_pool(name="ptp", bufs=8) as ptp,
            tc.tile_pool(name="work", bufs=2) as workp,
            tc.tile_pool(name="otsp", bufs=5) as otsp,
            tc.tile_pool(name="obp", bufs=2) as obp,
            tc.tile_pool(name="pp", bufs=2, space="PSUM") as pp,
            tc.tile_pool(name="pqk", bufs=2, space="PSUM") as pqk,
            tc.tile_pool(name="ppv", bufs=2, space="PSUM") as ppv,
        ):
            # ---------- constants into SBUF ----------
            wq_sb = constp.tile([128, HC * 2 * DH], dt.bfloat16, tag="wq")
            wk_sb = constp.tile([128, HC * DH], dt.bfloat16, tag="wk")
            wv_sb = constp.tile([128, HC * DH], dt.bfloat16, tag="wv")
            wo_sb = constp.tile([128, 2 * H], dt.bfloat16, tag="wo")
            bq_sb = constp.tile([128, 2], dt.float32, tag="bq")
            bkv_sb = constp.tile([128, 2], dt.float32, tag="bkv")
            cos_sb = constp.tile([128, S], dt.bfloat16, tag="cos")
            sin_sb = constp.tile([128, S], dt.bfloat16, tag="sin")
            msk_sb = constp.tile([128, 4 * W], dt.bfloat16, tag="msk")
            ones_sb = constp.tile([128, 1], dt.float16, tag="ones")
            ident = constp.tile([128, 128], dt.bfloat16, tag="ident")
            negC = constp.tile([128, 1], dt.float32, tag="negC")

            # startup: only the window-0 critical DMAs go on the sync queue;
            # everything else queues behind on gpsimd so it doesn't steal HBM
            # bandwidth from the first projection matmuls
            nc.sync.dma_start(wk_sb[:], wk[:, :])
            nc.sync.dma_start(wq_sb[:, 0:1024], wq[:, 0:1024])
            nc.gpsimd.dma_start(bq_sb[:], bqd[:, :])
            nc.gpsimd.dma_start(bkv_sb[:], bkvd[:, :])
            nc.gpsimd.dma_start(cos_sb[:], cosd[:, :])
            nc.gpsimd.dma_start(sin_sb[:], sind[:, :])
            nc.gpsimd.dma_start(wv_sb[:], wv[:, :])
            nc.gpsimd.memset(ones_sb[:], 1.0)
            nc.gpsimd.memset(negC[:], EXP_SHIFT)
            make_identity(nc, ident[:])

            qt_sb = projp.tile([128, 2 * S], dt.bfloat16, tag="qt")
            kt_q = [projp.tile([128, SQ], dt.bfloat16, tag=f"ktq{r}",
                               name=f"ktq{r}") for r in range(4)]
            vn_q = [projp.tile([128, SQ], dt.bfloat16, tag=f"vnq{r}",
                               name=f"vnq{r}") for r in range(4)]

            def kt_chunk(k):
                return kt_q[k // 8][:, (k % 8) * 128:(k % 8 + 1) * 128]

            def vn_chunk(k):
                return vn_q[k // 8][:, (k % 8) * 128:(k % 8 + 1) * 128]

            shuffle_mask = [i ^ 1 for i in range(32)]

            def rope_store(t0, dest_slc, sb):
                tsw = workp.tile([128, W], dt.bfloat16, tag="ropesw")
                nc.vector.stream_shuffle(tsw[:], t0[:], mask=shuffle_mask)
                t1 = workp.tile([128, W], dt.bfloat16, tag="rope1")
                nc.vector.tensor_mul(t1[:], t0[:],
                                     cos_sb[:, sb * W:(sb + 1) * W])
                t2 = workp.tile([128, W], dt.bfloat16, tag="rope2")
                nc.vector.tensor_mul(t2[:], tsw[:],
                                     sin_sb[:, sb * W:(sb + 1) * W])
                nc.vector.tensor_add(dest_slc, t1[:], t2[:])

            # ---------- projection filler units for window sb ----------
            # each unit emits ~2-4 PE matmuls (plus evac side-work on
            # ACT/DVE at target boundaries); DMA for xb is issued when the
            # unit list is built
            def proj_units(sb):
                xb = xtp.tile([128, HC * W], dt.bfloat16, tag="xtb")
                if sb == 0:
                    for q4 in range(4):
                        nc.sync.dma_start(
                            xb[:, q4 * HC * W // 4:(q4 + 1) * HC * W // 4],
                            xt[0, :, q4 * HC * W // 4:(q4 + 1) * HC * W // 4])
                    nc.sync.dma_start(wq_sb[:, 1024:], wq[:, 1024:])
                targets = [
                    ("rope", lambda h: wk_sb[:, h * 128:(h + 1) * 128],
                     bkv_sb[:, 0:1], kt_q[sb // 2], (sb % 2) * W),
                    ("rope", lambda h: wq_sb[:, h * 256:h * 256 + 128],
                     bq_sb[:, 0:1], qt_sb, sb * W),
                    ("vnat", lambda h: wv_sb[:, h * 128:(h + 1) * 128],
                     bkv_sb[:, 1:2], vn_q[sb // 2], 0),
                    ("rope", lambda h: wq_sb[:, h * 256 + 128:h * 256 + 256],
                     bq_sb[:, 1:2], qt_sb, S + sb * W),
                ]
                units = []
                state = {}

                def mm_unit(kind, wslc, bias, dest, doff, h0, first_unit):
                    def emit():
                        if first_unit and sb > 0:
                            nc.sync.dma_start(xb[:], xt[sb, :, :])
                        if h0 == 0:
                            state["ps"] = pp.tile([128, W], dt.float32,
                                                  tag="pp_ps", name="ps")
                        ps = state["ps"]
                        for h in range(h0, h0 + 4):
                            nc.tensor.matmul(
                                ps[:], wslc(h), xb[:, h * W:(h + 1) * W],
                                start=(h == 0), stop=(h == HC - 1))
                        if h0 + 4 == HC:
                            t0 = workp.tile([128, W], dt.bfloat16, tag="evac0",
                                            name="t0")
                            nc.scalar.activation(t0[:], ps[:], AF.Identity,
                                                 bias=bias)
                            if kind == "rope":
                                rope_store(t0, dest[:, doff:doff + W], sb)
                            else:
                                state["vstage"] = t0
                    return emit

                def tr_unit(dest, i):
                    def emit():
                        t0 = state["vstage"]
                        tp = pp.tile([128, 128], dt.bfloat16, tag="pp_ps",
                                     name="tp")
                        nc.tensor.transpose(
                            tp[:], t0[:, i * 128:(i + 1) * 128], ident[:])
                        nc.vector.tensor_copy(
                            dest[:, ((sb % 2) * 4 + i) * 128:
                                 ((sb % 2) * 4 + i + 1) * 128],
                            tp[:])
                    return emit

                for ti, (kind, wslc, bias, dest, doff) in enumerate(targets):
                    for h0 in range(0, HC, 4):
                        units.append(mm_unit(kind, wslc, bias, dest, doff, h0,
                                             ti == 0 and h0 == 0))
                    if kind == "vnat":
                        for i in range(W // 128):
                            units.append(tr_unit(dest, i))
                return units

            # ---------- o_proj filler units for window jm1 ----------
            def oproj_units(jm1, ots_heads, evac):
                units = []

                def qc_unit(qc, n):
                    def emit():
                        if n == 0:
                            oproj_state[qc] = obp.tile([128, H], dt.float32,
                                                       tag="ob", name="ob")
                        ob = oproj_state[qc]
                        po = pp.tile([128, W], dt.float32, tag="pp_ps",
                                     name="po")
                        for a in range(2):
                            nc.tensor.matmul(
                                po[:],
                                ots_heads[a][:, qc * 128:(qc + 1) * 128],
                                wo_sb[:, a * H + n * W: a * H + (n + 1) * W],
                                start=(a == 0), stop=(a == 1))
                        eng = evac if evac in ("act", "dve") else \
                            ("act" if n % 2 == 0 else "dve")
                        if eng == "act":
                            nc.scalar.activation(ob[:, n * W:(n + 1) * W],
                                                 po[:], AF.Copy)
                        else:
                            nc.vector.tensor_copy(ob[:, n * W:(n + 1) * W],
                                                  po[:])
                        if n == H // W - 1:
                            nc.sync.dma_start(
                                out[jm1 * W + qc * 128:
                                    jm1 * W + (qc + 1) * 128, :], ob[:])
                    return emit

                oproj_state = {}
                for qc in range(4):
                    for n in range(H // W):
                        units.append(qc_unit(qc, n))
                return units

            # ---------- attention for one head, with fillers ----------
            def attn_head(a, j, fillers):
                nkc = 4 * j + 4
                split = j >= 4  # two dacc accumulators for long chains
                qslc = qt_sb[:, a * S + j * W: a * S + (j + 1) * W]
                ot = ppv.tile([128, W], dt.float32, tag="ppv_ps", name="ot")
                dacc0 = workp.tile([128, 2 * W], dt.float16, tag="dacc0",
                                   name="dacc0")
                dacc1 = (workp.tile([128, 2 * W], dt.float16, tag="dacc1",
                                    name="dacc1") if split else None)
                for g in range(nkc // 2):
                    ps = pqk.tile([128, 2 * W], dt.float32, tag="qk_ps",
                                  name="ps")
                    ptg = ptp.tile([128, 2 * W], dt.bfloat16, tag="pt",
                                   name="ptg")
                    dacc = dacc1 if (split and g % 2 == 1) else dacc0
                    first = g < 2 if split else g < 1
                    last = g == nkc // 2 - 1
                    if last:
                        # QK only over the causally-valid columns (the rest
                        # of this psum is never read by the slimmed exp)
                        nc.tensor.matmul(
                            ps[:, 256:512], kt_chunk(2 * g),
                            qslc[:, 256:512], start=True, stop=True)
                        nc.tensor.matmul(
                            ps[:, 896:1024], kt_chunk(2 * g + 1),
                            qslc[:, 384:512], start=True, stop=True)
                    else:
                        for r in range(2):
                            nc.tensor.matmul(
                                ps[:, r * W:(r + 1) * W],
                                kt_chunk(2 * g + r),
                                qslc, start=True, stop=True)
                    for f in fillers.take():
                        f()
                    if last:
                        # last group = diagonal chunks r=2,3: columns
                        # [0:256] / [512:896] are fully causal-masked, so
                        # exp/mask/dacc/PV all skip them
                        nc.scalar.activation(ptg[:, 256:512], ps[:, 256:512],
                                             AF.Exp, scale=SCALE, bias=negC[:])
                        nc.scalar.activation(ptg[:, 896:1024], ps[:, 896:1024],
                                             AF.Exp, scale=SCALE, bias=negC[:])
                        nc.vector.tensor_mul(
                            ptg[:, 256:512], ptg[:, 256:512],
                            msk_sb[:, 2 * W + 256:3 * W])
                        nc.vector.tensor_mul(
                            ptg[:, 896:1024], ptg[:, 896:1024],
                            msk_sb[:, 3 * W + 384:4 * W])
                        nc.vector.tensor_add(dacc[:, 256:512],
                                             dacc[:, 256:512],
                                             ptg[:, 256:512])
                        nc.vector.tensor_add(dacc[:, 896:1024],
                                             dacc[:, 896:1024],
                                             ptg[:, 896:1024])
                        nc.tensor.matmul(
                            ot[:, 256:512], vn_chunk(2 * g),
                            ptg[:, 256:512], start=False, stop=False)
                        nc.tensor.matmul(
                            ot[:, 384:512], vn_chunk(2 * g + 1),
                            ptg[:, 896:1024], start=False, stop=True)
                        continue
                    nc.scalar.activation(ptg[:], ps[:], AF.Exp,
                                         scale=SCALE, bias=negC[:])
                    if g == nkc // 2 - 2:
                        nc.vector.tensor_mul(
                            ptg[:], ptg[:], msk_sb[:, 0:2 * W])
                    if first:
                        nc.vector.tensor_copy(dacc[:], ptg[:])
                    else:
                        nc.vector.tensor_add(dacc[:], dacc[:], ptg[:])
                    for r in range(2):
                        k = 2 * g + r
                        nc.tensor.matmul(
                            ot[:], vn_chunk(k),
                            ptg[:, r * W:(r + 1) * W],
                            start=(k == 0), stop=False)
                return ot, dacc0, dacc1

            # window-end denominator + scale for one head -> ots tile
            def finish_head(ot, dacc0, dacc1):
                dn = pp.tile([1, W], dt.float32, tag="pp_ps", name="dn")
                segs = [dacc0[:, 0:W], dacc0[:, W:2 * W]]
                if dacc1 is not None:
                    segs += [dacc1[:, 0:W], dacc1[:, W:2 * W]]
                for i, seg in enumerate(segs):
                    nc.tensor.matmul(dn[0:1, :], ones_sb[:, 0:1], seg,
                                     start=(i == 0), stop=(i == len(segs) - 1))
                drc = workp.tile([1, W], dt.float32, tag="drc")
                nc.vector.reciprocal_approx_fast(drc[:], dn[0:1, :])
                drb = workp.tile([128, W], dt.float32, tag="drb")
                nc.gpsimd.partition_broadcast(drb[:], drc[:])
                ots = otsp.tile([128, W], dt.bfloat16, tag="ots")
                nc.vector.tensor_mul(ots[:], ot[:], drb[:])
                return ots

            class Fillers:
                """Distributes filler units evenly over `take()` calls."""

                def __init__(self, units, ntakes):
                    self.units = units
                    self.ntakes = max(ntakes, 1)
                    self.taken = 0
                    self.pos = 0

                def take(self):
                    self.taken += 1
                    end = (len(self.units) * self.taken) // self.ntakes
                    u = self.units[self.pos:end]
                    self.pos = end
                    return u

                def rest(self):
                    u = self.units[self.pos:]
                    self.pos = len(self.units)
                    return u

            # ---------- fused window loop ----------
            # window 0's projections run as a straight block (attention
            # depends on them); window j then computes attention j with
            # proj(j+1) and o_proj(j-1) as PE fillers inside the group loop
            for f in proj_units(0):
                f()
            nc.gpsimd.dma_start(msk_sb[:], mskd[:, :])
            nc.gpsimd.dma_start(wo_sb[:], wo[:, :])
            prev = None
            for j in range(NW):
                units = []
                if j + 1 < NW:
                    units += proj_units(j + 1)
                if prev is not None:
                    units += oproj_units(j - 1, prev, "dve")
                fillers = Fillers(units, 2 * (2 * j + 2))
                h0 = attn_head(0, j, fillers)
                h1 = attn_head(1, j, fillers)
                for f in fillers.rest():
                    f()
                o0 = finish_head(*h0)
                o1 = finish_head(*h1)
                prev = (o0, o1)
            units_a = oproj_units(NW - 1, prev, "act")
            units_d = oproj_units(NW - 1, prev, "dve")
            for i in range(16):
                (units_a if (i // 4) % 2 == 0 else units_d)[i]()

    nc.compile()
    return nc


def _prep_inputs(x, cos, sin, Wq, bq, Wk, bk, Wv, bv, Wo):
    x = np.asarray(x, dtype=np.float32).reshape(S, H)
    cos = np.asarray(cos, dtype=np.float32).reshape(S, DH)
    sin = np.asarray(sin, dtype=np.float32).reshape(S, DH)

    xtT = x.T.astype(BF16)                       # [H, S]
    # blocked layout: [seq_block, partition, hid_chunk * W] so each block's
    # DMA is one fully-contiguous read
    xtb = np.ascontiguousarray(
        xtT.reshape(HC, 128, NW, W).transpose(2, 1, 0, 3).reshape(NW, 128, HC * W))

    # head-dim permutation: partition 2t <- dim t, partition 2t+1 <- dim t+64
    perm = np.empty(DH, np.int64)
    perm[0::2] = np.arange(64)
    perm[1::2] = np.arange(64) + 64

    cosT = np.ascontiguousarray(cos.T)          # [128, S]
    sinT = np.ascontiguousarray(sin.T)
    cosP = np.ascontiguousarray(cosT[perm]).astype(BF16)
    sinsP = np.empty_like(sinT)
    sinsP[0::2] = -sinT[:64]
    sinsP[1::2] = sinT[:64]
    sinsP = np.ascontiguousarray(sinsP).astype(BF16)

    # causal 0/1 masks for the 4 diagonal-band chunks of each 512-q window
    kk = np.arange(128)[:, None]
    qq = np.arange(W)[None, :]
    msk = np.concatenate(
        [(qq >= kk + 128 * r).astype(np.float32) for r in range(4)],
        axis=1).astype(BF16)

    Wq = np.asarray(Wq, np.float32)
    Wk = np.asarray(Wk, np.float32)
    Wv = np.asarray(Wv, np.float32)
    Wo = np.asarray(Wo, np.float32)
    bq = np.asarray(bq, np.float32)
    bk = np.asarray(bk, np.float32)
    bv = np.asarray(bv, np.float32)

    in_maps = []
    for c in range(N_CORES):
        kv = c // 4
        # q/k projections get the RoPE head-dim permutation applied to their
        # output columns (and biases); v/o stay in natural order
        wq_c = np.concatenate(
            [Wq[:, (2 * c + a) * DH:(2 * c + a + 1) * DH][:, perm]
             for a in range(2)], axis=1)
        wk_c = Wk[:, kv * DH:(kv + 1) * DH][:, perm]
        wv_c = Wv[:, kv * DH:(kv + 1) * DH]
        wo_c = Wo[2 * c * DH:(2 * c + 2) * DH, :]
        bq_c = np.stack(
            [bq[(2 * c + a) * DH:(2 * c + a + 1) * DH][perm] for a in range(2)],
            axis=1)
        bkv_c = np.stack(
            [bk[kv * DH:(kv + 1) * DH][perm], bv[kv * DH:(kv + 1) * DH]],
            axis=1)

        def wrearr(w):
            c = w.shape[0] // 128
            return np.ascontiguousarray(
                w.reshape(c, 128, -1).transpose(1, 0, 2).reshape(128, -1))

        in_maps.append({
            "xt": xtb,
            "wq": wrearr(wq_c).astype(BF16),
            "wk": wrearr(wk_c).astype(BF16),
            "wv": wrearr(wv_c).astype(BF16),
            "wo": wrearr(wo_c).astype(BF16),
            "bq": np.ascontiguousarray(bq_c).astype(np.float32),
            "bkv": np.ascontiguousarray(bkv_c).astype(np.float32),
            "cost": cosP, "sins": sinsP,
            "msk": msk,
        })
    return in_maps


def _get_nc():
    if "nc" not in _CACHE:
        _CACHE["nc"] = _build()
    return _CACHE["nc"]


def run(trace=False, tmpdir=None, **inputs):
    from concourse.bass_utils import run_bass_kernel_spmd

    nc = _get_nc()
    in_maps = _prep_inputs(**inputs)
    kw = {}
    if trace:
        kw = dict(trace=True, tmpdir=tmpdir)
    res = run_bass_kernel_spmd(nc, in_maps, core_ids=list(range(N_CORES)), **kw)
    acc = np.zeros((S, H), dtype=np.float32)
    for r in res.results:
        acc += r["out"]
    return acc.reshape(1, S, H), res


def kernel(**inputs) -> np.ndarray:
    out, _ = run(**inputs)
    return out
